# revision 8
# baseline (speedup 1.0000x reference)
"""Trainium2 Bass kernel for per-(b,v)-slice masked attention.

Reference computation (per (b,v) slice, P=S=512, D=512):
    q = X_q @ Wq.T + bq          (softmax scale folded into Wq/bq here)
    k = X_k @ Wkv.T + bkv
    v = X_v @ Wkv.T + bkv
    scores = q @ k.T, diag masked, attn = softmax(scores)
    out = (attn @ v) @ Wo.T + bo

Sharding: 128 (b,v) slices split 16-per-core across 8 cores; projections
replicated. Host pre-transposes activations to d-major layout so every
on-chip matmul contracts over the partition dimension.

On-chip dataflow per slice (all matmuls are lhsT.T @ rhs, contracting
over partitions):
    qT[o,p]  = (WqT tiles).T @ XqT      kT[o,s] likewise
    v[s,o]   = (XvT tiles).T @ WkvT     (natural layout)
    sT[s,p]  = (kT tiles).T @ qT        (scores transposed)
    eT[s,p]  = exp(sT) * (1 - I)        (diag mask, multiplicative)
    avT[o,p] = (v tiles).T @ eT         (unnormalized)
    sums[p]  = (eT tiles).T @ ones      (softmax denominator)
    out[p,o] = ((avT tiles).T @ WoT) * recip(sums)[p] + bo

Matmul operands use float32r (tfloat32): the PE streams them at 1
cycle/row vs 4 for fp32. All PSUM accumulation is fp32.
"""

import numpy as np

import concourse.bacc as bacc
import concourse.mybir as mybir
from concourse.tile import TileContext
from concourse.bass_utils import run_bass_kernel_spmd

B, V, P, D = 4, 32, 512, 512
N_CORES = 8
SLICES = B * V  # 128
SPC = SLICES // N_CORES  # 16 slices per core
KT = D // 128  # 4 contraction tiles
PT = P // 128  # 4 token tiles

USE_FP32R = True
HOST_ROUND_TF32 = True

F32 = mybir.dt.float32
R = mybir.dt.float32r if USE_FP32R else F32

AF = mybir.ActivationFunctionType
ALU = mybir.AluOpType


def build_program():
    """Build the SPMD Bass program (identical on all 8 cores)."""
    nc = bacc.Bacc("TRN2", target_bir_lowering=False, debug=False, num_devices=N_CORES)

    xq_d = nc.dram_tensor("xqT", [SPC, D, P], R, kind="ExternalInput")
    xk_d = nc.dram_tensor("xkT", [SPC, D, P], R, kind="ExternalInput")
    xv_d = nc.dram_tensor("xvT", [SPC, D, P], R, kind="ExternalInput")
    wq_d = nc.dram_tensor("wqT", [D, D], R, kind="ExternalInput")
    wkv_d = nc.dram_tensor("wkvT", [D, D], R, kind="ExternalInput")
    wo_d = nc.dram_tensor("woT", [D, D], R, kind="ExternalInput")
    bq_d = nc.dram_tensor("bq_col", [128, KT], F32, kind="ExternalInput")
    bkv_d = nc.dram_tensor("bkv_col", [128, KT], F32, kind="ExternalInput")
    bkvb_d = nc.dram_tensor("bkv_bc", [128, D], F32, kind="ExternalInput")
    bob_d = nc.dram_tensor("bo_bc", [128, D], F32, kind="ExternalInput")
    mask_d = nc.dram_tensor("mask", [128, 128], R, kind="ExternalInput")
    ones_d = nc.dram_tensor("ones", [128, 1], F32, kind="ExternalInput")
    out_d = nc.dram_tensor("out", [SPC, P, D], F32, kind="ExternalOutput")

    with TileContext(nc) as tc:
        with (
            tc.tile_pool(name="consts", bufs=1) as cpool,
            tc.tile_pool(name="xin", bufs=2) as xpool,
            tc.tile_pool(name="proj", bufs=2) as ppool,
            tc.tile_pool(name="attn", bufs=2) as apool,
            tc.tile_pool(name="outp", bufs=2) as opool,
            tc.tile_pool(name="small", bufs=2) as spool,
            tc.tile_pool(name="psum", bufs=6, space="PSUM") as mmpool,
            tc.tile_pool(name="psum_sums", bufs=2, space="PSUM") as sumpool,
        ):
            # ---- constants (loaded once) ----
            def load_w(dram):
                t = cpool.tile([128, KT, D], R, tag=dram.name)
                nc.sync.dma_start(
                    out=t[:], in_=dram.ap().rearrange("(kk p) f -> p kk f", p=128)
                )
                return t

            wq_sb = load_w(wq_d)
            wkv_sb = load_w(wkv_d)
            wo_sb = load_w(wo_d)
            bq_sb = cpool.tile([128, KT], F32, tag="bq")
            nc.sync.dma_start(out=bq_sb[:], in_=bq_d.ap())
            bkv_sb = cpool.tile([128, KT], F32, tag="bkv")
            nc.sync.dma_start(out=bkv_sb[:], in_=bkv_d.ap())
            bkvb_sb = cpool.tile([128, D], F32, tag="bkvb")
            nc.sync.dma_start(out=bkvb_sb[:], in_=bkvb_d.ap())
            bob_sb = cpool.tile([128, D], F32, tag="bob")
            nc.sync.dma_start(out=bob_sb[:], in_=bob_d.ap())
            mask_sb = cpool.tile([128, 128], R, tag="mask")
            nc.sync.dma_start(out=mask_sb[:], in_=mask_d.ap())
            ones_sb = cpool.tile([128, 1], F32, tag="ones")
            nc.sync.dma_start(out=ones_sb[:], in_=ones_d.ap())

            for s in range(SPC):
                # ---- load transposed activations ----
                xq = xpool.tile([128, KT, P], R, tag="xq")
                nc.sync.dma_start(
                    out=xq[:],
                    in_=xq_d.ap()[s].rearrange("(kk p) f -> p kk f", p=128),
                )
                xk = xpool.tile([128, KT, P], R, tag="xk")
                nc.sync.dma_start(
                    out=xk[:],
                    in_=xk_d.ap()[s].rearrange("(kk p) f -> p kk f", p=128),
                )
                xv = xpool.tile([128, KT, P], R, tag="xv")
                nc.sync.dma_start(
                    out=xv[:],
                    in_=xv_d.ap()[s].rearrange("(kk p) f -> p kk f", p=128),
                )

                # ---- projections ----
                qT = ppool.tile([128, KT, P], R, tag="qT")  # [dout, p]
                kTt = ppool.tile([128, KT, P], R, tag="kT")  # [dout, s]
                vn = ppool.tile([128, PT, D], R, tag="vn")  # [s, dout]
                for m in range(KT):
                    ps = mmpool.tile([128, P], F32, tag="mm")
                    for kk in range(KT):
                        nc.tensor.matmul(
                            ps[:], lhsT=wq_sb[:, kk, 128 * m : 128 * (m + 1)],
                            rhs=xq[:, kk, :], start=kk == 0, stop=kk == KT - 1)
                    nc.vector.tensor_scalar_add(qT[:, m, :], ps[:], bq_sb[:, m : m + 1])
                for m in range(KT):
                    ps = mmpool.tile([128, P], F32, tag="mm")
                    for kk in range(KT):
                        nc.tensor.matmul(
                            ps[:], lhsT=wkv_sb[:, kk, 128 * m : 128 * (m + 1)],
                            rhs=xk[:, kk, :], start=kk == 0, stop=kk == KT - 1)
                    nc.vector.tensor_scalar_add(kTt[:, m, :], ps[:], bkv_sb[:, m : m + 1])
                for i in range(PT):
                    ps = mmpool.tile([128, D], F32, tag="mm")
                    for kk in range(KT):
                        nc.tensor.matmul(
                            ps[:], lhsT=xv[:, kk, 128 * i : 128 * (i + 1)],
                            rhs=wkv_sb[:, kk, :], start=kk == 0, stop=kk == KT - 1)
                    nc.vector.tensor_add(vn[:, i, :], ps[:], bkvb_sb[:])

                # ---- scoresT + exp + diag mask ----
                eT = apool.tile([128, PT, P], R, tag="eT")  # [s, p]
                for i in range(PT):
                    ps = mmpool.tile([128, P], F32, tag="mm")
                    for kk in range(KT):
                        nc.tensor.matmul(
                            ps[:], lhsT=kTt[:, kk, 128 * i : 128 * (i + 1)],
                            rhs=qT[:, kk, :], start=kk == 0, stop=kk == KT - 1)
                    nc.scalar.activation(eT[:, i, :], ps[:], AF.Exp)
                    nc.vector.tensor_mul(
                        eT[:, i, 128 * i : 128 * (i + 1)],
                        eT[:, i, 128 * i : 128 * (i + 1)],
                        mask_sb[:],
                    )

                # ---- avT (unnormalized) ----
                avT = apool.tile([128, KT, P], R, tag="avT")  # [dv, p]
                for m in range(KT):
                    ps = mmpool.tile([128, P], F32, tag="mm")
                    for i in range(PT):
                        nc.tensor.matmul(
                            ps[:], lhsT=vn[:, i, 128 * m : 128 * (m + 1)],
                            rhs=eT[:, i, :], start=i == 0, stop=i == PT - 1)
                    nc.vector.tensor_copy(avT[:, m, :], ps[:])

                # ---- softmax denominators ----
                ps_sum = sumpool.tile([128, PT], F32, tag="sums")
                # N=1 matmuls violate fp32r's even-free-dim ISA rule; the
                # floor-cost denominator matmuls run as plain fp32 instead.
                for j in range(PT):
                    for i in range(PT):
                        nc.tensor.matmul(
                            ps_sum[:, j : j + 1],
                            lhsT=eT[:, i, 128 * j : 128 * (j + 1)].bitcast(F32),
                            rhs=ones_sb[:], start=i == 0, stop=i == PT - 1)
                recip = spool.tile([128, PT], F32, tag="recip")
                nc.vector.reciprocal(recip[:], ps_sum[:])

                # ---- output projection + normalize + bias ----
                ot = opool.tile([128, PT, D], F32, tag="ot")
                for j in range(PT):
                    ps = mmpool.tile([128, D], F32, tag="mm")
                    for m in range(KT):
                        nc.tensor.matmul(
                            ps[:], lhsT=avT[:, m, 128 * j : 128 * (j + 1)],
                            rhs=wo_sb[:, m, :], start=m == 0, stop=m == KT - 1)
                    nc.vector.scalar_tensor_tensor(
                        ot[:, j, :], ps[:], recip[:, j : j + 1], bob_sb[:],
                        ALU.mult, ALU.add,
                    )
                nc.sync.dma_start(
                    out=out_d.ap()[s].rearrange("(j p) f -> p j f", p=128),
                    in_=ot[:],
                )

    nc.compile()
    return nc


def _round_tf32(a):
    """Round fp32 to tf32 (10-bit mantissa) with round-to-nearest-even."""
    u = a.view(np.uint32).astype(np.uint64)
    u = (u + 0xFFF + ((u >> 13) & 1)) & 0xFFFFE000
    return u.astype(np.uint32).view(np.float32)


def prep_in_maps(queries, keys, values, Wq, bq, Wkv, bkv, Wo, bo):
    queries = np.asarray(queries, np.float32).reshape(SLICES, P, D)
    keys = np.asarray(keys, np.float32).reshape(SLICES, P, D)
    values = np.asarray(values, np.float32).reshape(SLICES, P, D)
    Wq = np.asarray(Wq, np.float32)
    Wkv = np.asarray(Wkv, np.float32)
    Wo = np.asarray(Wo, np.float32)
    bq = np.asarray(bq, np.float32)
    bkv = np.asarray(bkv, np.float32)
    bo = np.asarray(bo, np.float32)

    scale = np.float32(1.0 / np.sqrt(D))
    wqT = np.ascontiguousarray((Wq * scale).T)  # [din, dout]
    wkvT = np.ascontiguousarray(Wkv.T)
    woT = np.ascontiguousarray(Wo.T)
    bq_col = np.ascontiguousarray((bq * scale).reshape(KT, 128).T)
    bkv_col = np.ascontiguousarray(bkv.reshape(KT, 128).T)
    bkv_bc = np.ascontiguousarray(np.broadcast_to(bkv, (128, D)))
    bo_bc = np.ascontiguousarray(np.broadcast_to(bo, (128, D)))
    mask = (1.0 - np.eye(128)).astype(np.float32)

    qT = np.ascontiguousarray(queries.transpose(0, 2, 1))  # [slices, D, P]
    kT = np.ascontiguousarray(keys.transpose(0, 2, 1))
    vT = np.ascontiguousarray(values.transpose(0, 2, 1))

    if USE_FP32R and HOST_ROUND_TF32:
        qT, kT, vT = _round_tf32(qT), _round_tf32(kT), _round_tf32(vT)
        wqT, wkvT, woT = _round_tf32(wqT), _round_tf32(wkvT), _round_tf32(woT)

    in_maps = []
    for c in range(N_CORES):
        sl = slice(c * SPC, (c + 1) * SPC)
        in_maps.append({
            "xqT": qT[sl],
            "xkT": kT[sl],
            "xvT": vT[sl],
            "wqT": wqT,
            "wkvT": wkvT,
            "woT": woT,
            "bq_col": bq_col,
            "bkv_col": bkv_col,
            "bkv_bc": bkv_bc,
            "bo_bc": bo_bc,
            "mask": mask,
            "ones": np.ones((128, 1), np.float32),
        })
    return in_maps


_nc_cache = None


def kernel(**inputs):
    global _nc_cache
    if _nc_cache is None:
        _nc_cache = build_program()
    nc = _nc_cache
    in_maps = prep_in_maps(**inputs)
    res = run_bass_kernel_spmd(nc, in_maps, core_ids=list(range(N_CORES)))
    out = np.concatenate([res.results[c]["out"] for c in range(N_CORES)], axis=0)
    return out.reshape(B, V, P, D)


# revision 9
# speedup vs baseline: 1.0248x; 1.0248x over previous
"""Trainium2 Bass kernel for per-(b,v)-slice masked attention.

Reference computation (per (b,v) slice, P=S=512, D=512):
    q = X_q @ Wq.T + bq          (softmax scale folded into Wq/bq here)
    k = X_k @ Wkv.T + bkv
    v = X_v @ Wkv.T + bkv
    scores = q @ k.T, diag masked, attn = softmax(scores)
    out = (attn @ v) @ Wo.T + bo

Sharding: 128 (b,v) slices split 16-per-core across 8 cores; projections
replicated. Host pre-transposes activations to d-major layout so every
on-chip matmul contracts over the partition dimension.

On-chip dataflow per slice (all matmuls are lhsT.T @ rhs, contracting
over partitions):
    qT[o,p]  = (WqT tiles).T @ XqT      kT[o,s] likewise
    v[s,o]   = (XvT tiles).T @ WkvT     (natural layout)
    sT[s,p]  = (kT tiles).T @ qT        (scores transposed)
    eT[s,p]  = exp(sT) * (1 - I)        (diag mask, multiplicative)
    avT[o,p] = (v tiles).T @ eT         (unnormalized)
    row[1,p] = (ones tile).T @ eT       (softmax denominator, in psum row 0)
    rcpT     = tiny K=1 matmuls transposing recip(row) to per-partition
    out[p,o] = ((avT tiles).T @ WoT) * rcpT[p] + bo

Matmul operands are bf16 (PE streams 1 cycle/row with fast weight load);
all PSUM accumulation and the softmax normalization stay fp32.
"""

import numpy as np
import ml_dtypes

import concourse.bacc as bacc
import concourse.mybir as mybir
from concourse.tile import TileContext
from concourse.bass_utils import run_bass_kernel_spmd

B, V, P, D = 4, 32, 512, 512
N_CORES = 8
SLICES = B * V  # 128
SPC = SLICES // N_CORES  # 16 slices per core
KT = D // 128  # 4 contraction tiles
PT = P // 128  # 4 token tiles

# "bf16" | "fp32r" | "fp32" — matmul operand precision (PSUM is always fp32)
MM_MODE = "bf16"

F32 = mybir.dt.float32
R = {"bf16": mybir.dt.bfloat16, "fp32r": mybir.dt.float32r, "fp32": F32}[MM_MODE]
NP_R = {"bf16": ml_dtypes.bfloat16, "fp32r": np.float32, "fp32": np.float32}[MM_MODE]

AF = mybir.ActivationFunctionType
ALU = mybir.AluOpType


def build_program():
    """Build the SPMD Bass program (identical on all 8 cores)."""
    nc = bacc.Bacc("TRN2", target_bir_lowering=False, debug=False, num_devices=N_CORES)

    xq_d = nc.dram_tensor("xqT", [SPC, D, P], R, kind="ExternalInput")
    xk_d = nc.dram_tensor("xkT", [SPC, D, P], R, kind="ExternalInput")
    xv_d = nc.dram_tensor("xvT", [SPC, D, P], R, kind="ExternalInput")
    wq_d = nc.dram_tensor("wqT", [D, D], R, kind="ExternalInput")
    wkv_d = nc.dram_tensor("wkvT", [D, D], R, kind="ExternalInput")
    wo_d = nc.dram_tensor("woT", [D, D], R, kind="ExternalInput")
    bq_d = nc.dram_tensor("bq_col", [128, KT], F32, kind="ExternalInput")
    bkv_d = nc.dram_tensor("bkv_col", [128, KT], F32, kind="ExternalInput")
    bkvb_d = nc.dram_tensor("bkv_bc", [128, D], F32, kind="ExternalInput")
    bob_d = nc.dram_tensor("bo_bc", [128, D], F32, kind="ExternalInput")
    mask_d = nc.dram_tensor("mask", [128, 128], R, kind="ExternalInput")
    # columns: [1, 0] — column 0 computes the row-sum, column 1 pads the
    # stationary free dim to an even count (fp32r ISA requirement)
    ones_d = nc.dram_tensor("ones2", [128, 2], R, kind="ExternalInput")
    one1_d = nc.dram_tensor("one1", [1, 1], F32, kind="ExternalInput")
    out_d = nc.dram_tensor("out", [SPC, P, D], F32, kind="ExternalOutput")

    with TileContext(nc) as tc:
        with (
            tc.tile_pool(name="consts", bufs=1) as cpool,
            tc.tile_pool(name="xin", bufs=2) as xpool,
            tc.tile_pool(name="proj", bufs=2) as ppool,
            tc.tile_pool(name="attn", bufs=2) as apool,
            tc.tile_pool(name="outp", bufs=2) as opool,
            tc.tile_pool(name="small", bufs=2) as spool,
            tc.tile_pool(name="psum", bufs=4, space="PSUM") as mmpool,
            tc.tile_pool(name="psum_row", bufs=2, space="PSUM") as rowpool,
            tc.tile_pool(name="psum_rt", bufs=2, space="PSUM") as rtpool,
        ):
            # ---- constants (loaded once) ----
            def load_w(dram):
                t = cpool.tile([128, KT, D], R, tag=dram.name)
                nc.sync.dma_start(
                    out=t[:], in_=dram.ap().rearrange("(kk p) f -> p kk f", p=128)
                )
                return t

            wq_sb = load_w(wq_d)
            wkv_sb = load_w(wkv_d)
            wo_sb = load_w(wo_d)
            bq_sb = cpool.tile([128, KT], F32, tag="bq")
            nc.sync.dma_start(out=bq_sb[:], in_=bq_d.ap())
            bkv_sb = cpool.tile([128, KT], F32, tag="bkv")
            nc.sync.dma_start(out=bkv_sb[:], in_=bkv_d.ap())
            bkvb_sb = cpool.tile([128, D], F32, tag="bkvb")
            nc.sync.dma_start(out=bkvb_sb[:], in_=bkvb_d.ap())
            bob_sb = cpool.tile([128, D], F32, tag="bob")
            nc.sync.dma_start(out=bob_sb[:], in_=bob_d.ap())
            mask_sb = cpool.tile([128, 128], R, tag="mask")
            nc.sync.dma_start(out=mask_sb[:], in_=mask_d.ap())
            ones_sb = cpool.tile([128, 2], R, tag="ones2")
            nc.sync.dma_start(out=ones_sb[:], in_=ones_d.ap())
            one1_sb = cpool.tile([1, 1], F32, tag="one1")
            nc.sync.dma_start(out=one1_sb[:], in_=one1_d.ap())

            for s in range(SPC):
                # ---- load transposed activations ----
                xq = xpool.tile([128, KT, P], R, tag="xq")
                nc.sync.dma_start(
                    out=xq[:],
                    in_=xq_d.ap()[s].rearrange("(kk p) f -> p kk f", p=128),
                )
                xk = xpool.tile([128, KT, P], R, tag="xk")
                nc.sync.dma_start(
                    out=xk[:],
                    in_=xk_d.ap()[s].rearrange("(kk p) f -> p kk f", p=128),
                )
                xv = xpool.tile([128, KT, P], R, tag="xv")
                nc.sync.dma_start(
                    out=xv[:],
                    in_=xv_d.ap()[s].rearrange("(kk p) f -> p kk f", p=128),
                )

                # ---- projections ----
                qT = ppool.tile([128, KT, P], R, tag="qT")  # [dout, p]
                kTt = ppool.tile([128, KT, P], R, tag="kT")  # [dout, s]
                vn = ppool.tile([128, PT, D], R, tag="vn")  # [s, dout]
                for m in range(KT):
                    ps = mmpool.tile([128, P], F32, tag="mm")
                    for kk in range(KT):
                        nc.tensor.matmul(
                            ps[:], lhsT=wq_sb[:, kk, 128 * m : 128 * (m + 1)],
                            rhs=xq[:, kk, :], start=kk == 0, stop=kk == KT - 1)
                    nc.scalar.activation(qT[:, m, :], ps[:], AF.Identity,
                                         bias=bq_sb[:, m : m + 1])
                for m in range(KT):
                    ps = mmpool.tile([128, P], F32, tag="mm")
                    for kk in range(KT):
                        nc.tensor.matmul(
                            ps[:], lhsT=wkv_sb[:, kk, 128 * m : 128 * (m + 1)],
                            rhs=xk[:, kk, :], start=kk == 0, stop=kk == KT - 1)
                    nc.scalar.activation(kTt[:, m, :], ps[:], AF.Identity,
                                         bias=bkv_sb[:, m : m + 1])
                for i in range(PT):
                    ps = mmpool.tile([128, D], F32, tag="mm")
                    for kk in range(KT):
                        nc.tensor.matmul(
                            ps[:], lhsT=xv[:, kk, 128 * i : 128 * (i + 1)],
                            rhs=wkv_sb[:, kk, :], start=kk == 0, stop=kk == KT - 1)
                    nc.vector.tensor_add(vn[:, i, :], ps[:], bkvb_sb[:])

                # ---- scoresT + exp + diag mask ----
                eT = apool.tile([128, PT, P], R, tag="eT")  # [s, p]
                for i in range(PT):
                    ps = mmpool.tile([128, P], F32, tag="mm")
                    for kk in range(KT):
                        nc.tensor.matmul(
                            ps[:], lhsT=kTt[:, kk, 128 * i : 128 * (i + 1)],
                            rhs=qT[:, kk, :], start=kk == 0, stop=kk == KT - 1)
                    nc.scalar.activation(eT[:, i, :], ps[:], AF.Exp)
                    nc.vector.tensor_mul(
                        eT[:, i, 128 * i : 128 * (i + 1)],
                        eT[:, i, 128 * i : 128 * (i + 1)],
                        mask_sb[:],
                    )

                # ---- softmax denominators: row-sums then transpose ----
                ps_row = rowpool.tile([2, P], F32, tag="row")
                for i in range(PT):
                    nc.tensor.matmul(
                        ps_row[:], lhsT=ones_sb[:], rhs=eT[:, i, :],
                        start=i == 0, stop=i == PT - 1)
                rrow = spool.tile([1, P], F32, tag="rrow")
                nc.vector.reciprocal(rrow[:], ps_row[0:1, :])
                ps_rt = rtpool.tile([128, PT], F32, tag="rt")
                for j in range(PT):
                    nc.tensor.matmul(
                        ps_rt[:, j : j + 1],
                        lhsT=rrow[0:1, 128 * j : 128 * (j + 1)],
                        rhs=one1_sb[:], start=True, stop=True)
                rcpT = spool.tile([128, PT], F32, tag="rcpT")
                nc.vector.tensor_copy(rcpT[:], ps_rt[:])

                # ---- avT (unnormalized) ----
                avT = apool.tile([128, KT, P], R, tag="avT")  # [dv, p]
                for m in range(KT):
                    ps = mmpool.tile([128, P], F32, tag="mm")
                    for i in range(PT):
                        nc.tensor.matmul(
                            ps[:], lhsT=vn[:, i, 128 * m : 128 * (m + 1)],
                            rhs=eT[:, i, :], start=i == 0, stop=i == PT - 1)
                    nc.scalar.copy(avT[:, m, :], ps[:])

                # ---- output projection + normalize + bias ----
                ot = opool.tile([128, PT, D], F32, tag="ot")
                for j in range(PT):
                    ps = mmpool.tile([128, D], F32, tag="mm")
                    for m in range(KT):
                        nc.tensor.matmul(
                            ps[:], lhsT=avT[:, m, 128 * j : 128 * (j + 1)],
                            rhs=wo_sb[:, m, :], start=m == 0, stop=m == KT - 1)
                    nc.vector.scalar_tensor_tensor(
                        ot[:, j, :], ps[:], rcpT[:, j : j + 1], bob_sb[:],
                        ALU.mult, ALU.add,
                    )
                nc.sync.dma_start(
                    out=out_d.ap()[s].rearrange("(j p) f -> p j f", p=128),
                    in_=ot[:],
                )

    nc.compile()
    return nc


def _round_tf32(a):
    """Round fp32 to tf32 (10-bit mantissa) with round-to-nearest-even."""
    u = a.view(np.uint32).astype(np.uint64)
    u = (u + 0xFFF + ((u >> 13) & 1)) & 0xFFFFE000
    return u.astype(np.uint32).view(np.float32)


def _to_r(a):
    if MM_MODE == "bf16":
        return a.astype(ml_dtypes.bfloat16)
    if MM_MODE == "fp32r":
        return _round_tf32(a)
    return a


def prep_in_maps(queries, keys, values, Wq, bq, Wkv, bkv, Wo, bo):
    queries = np.asarray(queries, np.float32).reshape(SLICES, P, D)
    keys = np.asarray(keys, np.float32).reshape(SLICES, P, D)
    values = np.asarray(values, np.float32).reshape(SLICES, P, D)
    Wq = np.asarray(Wq, np.float32)
    Wkv = np.asarray(Wkv, np.float32)
    Wo = np.asarray(Wo, np.float32)
    bq = np.asarray(bq, np.float32)
    bkv = np.asarray(bkv, np.float32)
    bo = np.asarray(bo, np.float32)

    scale = np.float32(1.0 / np.sqrt(D))
    wqT = _to_r(np.ascontiguousarray((Wq * scale).T))  # [din, dout]
    wkvT = _to_r(np.ascontiguousarray(Wkv.T))
    woT = _to_r(np.ascontiguousarray(Wo.T))
    bq_col = np.ascontiguousarray((bq * scale).reshape(KT, 128).T)
    bkv_col = np.ascontiguousarray(bkv.reshape(KT, 128).T)
    bkv_bc = np.ascontiguousarray(np.broadcast_to(bkv, (128, D)))
    bo_bc = np.ascontiguousarray(np.broadcast_to(bo, (128, D)))
    mask = _to_r((1.0 - np.eye(128)).astype(np.float32))
    ones2 = np.zeros((128, 2), np.float32)
    ones2[:, 0] = 1.0
    ones2 = _to_r(ones2)

    qT = _to_r(np.ascontiguousarray(queries.transpose(0, 2, 1)))  # [slices, D, P]
    kT = _to_r(np.ascontiguousarray(keys.transpose(0, 2, 1)))
    vT = _to_r(np.ascontiguousarray(values.transpose(0, 2, 1)))

    in_maps = []
    for c in range(N_CORES):
        sl = slice(c * SPC, (c + 1) * SPC)
        in_maps.append({
            "xqT": qT[sl],
            "xkT": kT[sl],
            "xvT": vT[sl],
            "wqT": wqT,
            "wkvT": wkvT,
            "woT": woT,
            "bq_col": bq_col,
            "bkv_col": bkv_col,
            "bkv_bc": bkv_bc,
            "bo_bc": bo_bc,
            "mask": mask,
            "ones2": ones2,
            "one1": np.ones((1, 1), np.float32),
        })
    return in_maps


_nc_cache = None


def kernel(**inputs):
    global _nc_cache
    if _nc_cache is None:
        _nc_cache = build_program()
    nc = _nc_cache
    in_maps = prep_in_maps(**inputs)
    res = run_bass_kernel_spmd(nc, in_maps, core_ids=list(range(N_CORES)))
    out = np.concatenate([res.results[c]["out"] for c in range(N_CORES)], axis=0)
    return out.reshape(B, V, P, D)


# revision 10
# speedup vs baseline: 1.8484x; 1.8037x over previous
"""Trainium2 Bass kernel for per-(b,v)-slice masked attention.

Reference computation (per (b,v) slice, P=S=512, D=512):
    q = X_q @ Wq.T + bq          (softmax scale folded into Wq here)
    k = X_k @ Wkv.T + bkv
    v = X_v @ Wkv.T + bkv
    scores = q @ k.T, diag masked, attn = softmax(scores)
    out = (attn @ v) @ Wo.T + bo

Sharding: 128 (b,v) slices split 16-per-core across 8 cores; weights
replicated. The host pre-transposes q/k activations to d-major layout so
every on-chip matmul contracts over the partition dimension.

Fast path (bq == bkv == 0, which setup_inputs guarantees): fold the
weight products on the host —
    M  = (scale*Wq).T @ Wkv     so  scores.T = Xk @ M.T @ Xq.T
    N0 = Wkv.T @ Wo.T           so  out = (attn @ Xv) @ N0 + (Wo@bkv + bo)
eliminating the k and v projections (4 big matmul groups per slice
instead of 6). Per slice:
    u[d,p]    = (M tiles).T @ XqT
    sT[s,p]   = (XkT tiles).T @ u       (scores transposed)
    eT[s,p]   = exp(sT) * (1 - I)      (diag mask, multiplicative)
    sums[p]   = (eT tiles).T @ ones    (softmax denominator, [128,4] psum)
    axT[d,p]  = (Xv tiles).T @ eT      (attn @ Xv, transposed, unnormalized)
    out[p,o]  = ((axT tiles).T @ N0) * recip(sums)[p] + bo2

General path (nonzero bq/bkv): explicit q/k/v projections as above.

Matmul operands are bf16 (PE streams 1 cycle/row with fast weight load);
all PSUM accumulation and softmax normalization stay fp32.
"""

import numpy as np
import ml_dtypes

import concourse.bacc as bacc
import concourse.mybir as mybir
from concourse.tile import TileContext
from concourse.bass_utils import run_bass_kernel_spmd

B, V, P, D = 4, 32, 512, 512
N_CORES = 8
SLICES = B * V  # 128
SPC = SLICES // N_CORES  # 16 slices per core
KT = D // 128  # 4 contraction tiles
PT = P // 128  # 4 token tiles

BF16 = mybir.dt.bfloat16
F32 = mybir.dt.float32
AF = mybir.ActivationFunctionType
ALU = mybir.AluOpType


def _new_nc():
    return bacc.Bacc("TRN2", target_bir_lowering=False, debug=False,
                     num_devices=N_CORES)


def _load_w(nc, cpool, dram):
    t = cpool.tile([128, KT, D], BF16, tag=dram.name)
    nc.sync.dma_start(out=t[:], in_=dram.ap().rearrange("(kk p) f -> p kk f", p=128))
    return t


def _load_x(nc, xpool, dram, s, tag):
    t = xpool.tile([128, KT, P], BF16, tag=tag)
    nc.sync.dma_start(out=t[:], in_=dram.ap()[s].rearrange("(kk p) f -> p kk f", p=128))
    return t


def build_program_fast():
    """Zero-bias fast path: 4 matmul groups per slice."""
    nc = _new_nc()

    xq_d = nc.dram_tensor("xqT", [SPC, D, P], BF16, kind="ExternalInput")
    xk_d = nc.dram_tensor("xkT", [SPC, D, P], BF16, kind="ExternalInput")
    xv_d = nc.dram_tensor("xvN", [SPC, P, D], BF16, kind="ExternalInput")
    m_d = nc.dram_tensor("Mh", [D, D], BF16, kind="ExternalInput")
    n0_d = nc.dram_tensor("N0h", [D, D], BF16, kind="ExternalInput")
    bo2_d = nc.dram_tensor("bo2_bc", [128, D], F32, kind="ExternalInput")
    mask_d = nc.dram_tensor("mask", [128, 128], BF16, kind="ExternalInput")
    ones_d = nc.dram_tensor("ones1", [128, 1], BF16, kind="ExternalInput")
    out_d = nc.dram_tensor("out", [SPC, P, D], F32, kind="ExternalOutput")

    with TileContext(nc) as tc:
        with (
            tc.tile_pool(name="consts", bufs=1) as cpool,
            tc.tile_pool(name="xin", bufs=2) as xpool,
            tc.tile_pool(name="proj", bufs=2) as ppool,
            tc.tile_pool(name="attn", bufs=2) as apool,
            tc.tile_pool(name="outp", bufs=2) as opool,
            tc.tile_pool(name="small", bufs=2) as spool,
            tc.tile_pool(name="psum", bufs=6, space="PSUM") as mmpool,
            tc.tile_pool(name="psum_sums", bufs=2, space="PSUM") as sumpool,
        ):
            m_sb = _load_w(nc, cpool, m_d)
            n0_sb = _load_w(nc, cpool, n0_d)
            bo2_sb = cpool.tile([128, D], F32, tag="bo2")
            nc.sync.dma_start(out=bo2_sb[:], in_=bo2_d.ap())
            mask_sb = cpool.tile([128, 128], BF16, tag="mask")
            nc.sync.dma_start(out=mask_sb[:], in_=mask_d.ap())
            ones_sb = cpool.tile([128, 1], BF16, tag="ones1")
            nc.sync.dma_start(out=ones_sb[:], in_=ones_d.ap())

            for s in range(SPC):
                xq = _load_x(nc, xpool, xq_d, s, "xq")
                xk = _load_x(nc, xpool, xk_d, s, "xk")
                xv = _load_x(nc, xpool, xv_d, s, "xv")  # natural [s, d] tiles

                # ---- u = M.T-tiles @ XqT ----
                u = ppool.tile([128, KT, P], BF16, tag="u")  # [d1, p]
                for m in range(KT):
                    ps = mmpool.tile([128, P], F32, tag="mm")
                    for kk in range(KT):
                        nc.tensor.matmul(
                            ps[:], lhsT=m_sb[:, kk, 128 * m : 128 * (m + 1)],
                            rhs=xq[:, kk, :], start=kk == 0, stop=kk == KT - 1)
                    nc.scalar.copy(u[:, m, :], ps[:])

                # ---- scoresT + exp + diag mask ----
                eT = apool.tile([128, PT, P], BF16, tag="eT")  # [s, p]
                for i in range(PT):
                    ps = mmpool.tile([128, P], F32, tag="mm")
                    for kk in range(KT):
                        nc.tensor.matmul(
                            ps[:], lhsT=xk[:, kk, 128 * i : 128 * (i + 1)],
                            rhs=u[:, kk, :], start=kk == 0, stop=kk == KT - 1)
                    nc.scalar.activation(eT[:, i, :], ps[:], AF.Exp)
                    nc.vector.tensor_mul(
                        eT[:, i, 128 * i : 128 * (i + 1)],
                        eT[:, i, 128 * i : 128 * (i + 1)],
                        mask_sb[:],
                    )

                # ---- softmax denominators ----
                ps_sum = sumpool.tile([128, PT], F32, tag="sums")
                for j in range(PT):
                    for i in range(PT):
                        nc.tensor.matmul(
                            ps_sum[:, j : j + 1],
                            lhsT=eT[:, i, 128 * j : 128 * (j + 1)],
                            rhs=ones_sb[:], start=i == 0, stop=i == PT - 1)
                rcpT = spool.tile([128, PT], F32, tag="rcpT")
                nc.vector.reciprocal(rcpT[:], ps_sum[:])

                # ---- axT = attn @ Xv, transposed (unnormalized) ----
                axT = apool.tile([128, KT, P], BF16, tag="axT")  # [d, p]
                for m in range(KT):
                    ps = mmpool.tile([128, P], F32, tag="mm")
                    for i in range(PT):
                        nc.tensor.matmul(
                            ps[:], lhsT=xv[:, i, 128 * m : 128 * (m + 1)],
                            rhs=eT[:, i, :], start=i == 0, stop=i == PT - 1)
                    nc.scalar.copy(axT[:, m, :], ps[:])

                # ---- final projection + normalize + bias ----
                ot = opool.tile([128, PT, D], F32, tag="ot")
                for j in range(PT):
                    ps = mmpool.tile([128, D], F32, tag="mm")
                    for m in range(KT):
                        nc.tensor.matmul(
                            ps[:], lhsT=axT[:, m, 128 * j : 128 * (j + 1)],
                            rhs=n0_sb[:, m, :], start=m == 0, stop=m == KT - 1)
                    nc.vector.scalar_tensor_tensor(
                        ot[:, j, :], ps[:], rcpT[:, j : j + 1], bo2_sb[:],
                        ALU.mult, ALU.add,
                    )
                nc.sync.dma_start(
                    out=out_d.ap()[s].rearrange("(j p) f -> p j f", p=128),
                    in_=ot[:],
                )

    nc.compile()
    return nc


def build_program_general():
    """General path with explicit q/k/v projections (nonzero biases)."""
    nc = _new_nc()

    xq_d = nc.dram_tensor("xqT", [SPC, D, P], BF16, kind="ExternalInput")
    xk_d = nc.dram_tensor("xkT", [SPC, D, P], BF16, kind="ExternalInput")
    xv_d = nc.dram_tensor("xvT", [SPC, D, P], BF16, kind="ExternalInput")
    wq_d = nc.dram_tensor("wqT", [D, D], BF16, kind="ExternalInput")
    wkv_d = nc.dram_tensor("wkvT", [D, D], BF16, kind="ExternalInput")
    wo_d = nc.dram_tensor("woT", [D, D], BF16, kind="ExternalInput")
    bq_d = nc.dram_tensor("bq_col", [128, KT], F32, kind="ExternalInput")
    bkv_d = nc.dram_tensor("bkv_col", [128, KT], F32, kind="ExternalInput")
    bkvb_d = nc.dram_tensor("bkv_bc", [128, D], F32, kind="ExternalInput")
    bob_d = nc.dram_tensor("bo_bc", [128, D], F32, kind="ExternalInput")
    mask_d = nc.dram_tensor("mask", [128, 128], BF16, kind="ExternalInput")
    ones_d = nc.dram_tensor("ones1", [128, 1], BF16, kind="ExternalInput")
    out_d = nc.dram_tensor("out", [SPC, P, D], F32, kind="ExternalOutput")

    with TileContext(nc) as tc:
        with (
            tc.tile_pool(name="consts", bufs=1) as cpool,
            tc.tile_pool(name="xin", bufs=2) as xpool,
            tc.tile_pool(name="proj", bufs=2) as ppool,
            tc.tile_pool(name="attn", bufs=2) as apool,
            tc.tile_pool(name="outp", bufs=2) as opool,
            tc.tile_pool(name="small", bufs=2) as spool,
            tc.tile_pool(name="psum", bufs=6, space="PSUM") as mmpool,
            tc.tile_pool(name="psum_sums", bufs=2, space="PSUM") as sumpool,
        ):
            wq_sb = _load_w(nc, cpool, wq_d)
            wkv_sb = _load_w(nc, cpool, wkv_d)
            wo_sb = _load_w(nc, cpool, wo_d)
            bq_sb = cpool.tile([128, KT], F32, tag="bq")
            nc.sync.dma_start(out=bq_sb[:], in_=bq_d.ap())
            bkv_sb = cpool.tile([128, KT], F32, tag="bkv")
            nc.sync.dma_start(out=bkv_sb[:], in_=bkv_d.ap())
            bkvb_sb = cpool.tile([128, D], F32, tag="bkvb")
            nc.sync.dma_start(out=bkvb_sb[:], in_=bkvb_d.ap())
            bob_sb = cpool.tile([128, D], F32, tag="bob")
            nc.sync.dma_start(out=bob_sb[:], in_=bob_d.ap())
            mask_sb = cpool.tile([128, 128], BF16, tag="mask")
            nc.sync.dma_start(out=mask_sb[:], in_=mask_d.ap())
            ones_sb = cpool.tile([128, 1], BF16, tag="ones1")
            nc.sync.dma_start(out=ones_sb[:], in_=ones_d.ap())

            for s in range(SPC):
                xq = _load_x(nc, xpool, xq_d, s, "xq")
                xk = _load_x(nc, xpool, xk_d, s, "xk")
                xv = _load_x(nc, xpool, xv_d, s, "xv")

                qT = ppool.tile([128, KT, P], BF16, tag="qT")  # [dout, p]
                kTt = ppool.tile([128, KT, P], BF16, tag="kT")  # [dout, s]
                vn = ppool.tile([128, PT, D], BF16, tag="vn")  # [s, dout]
                for m in range(KT):
                    ps = mmpool.tile([128, P], F32, tag="mm")
                    for kk in range(KT):
                        nc.tensor.matmul(
                            ps[:], lhsT=wq_sb[:, kk, 128 * m : 128 * (m + 1)],
                            rhs=xq[:, kk, :], start=kk == 0, stop=kk == KT - 1)
                    nc.scalar.activation(qT[:, m, :], ps[:], AF.Identity,
                                         bias=bq_sb[:, m : m + 1])
                for m in range(KT):
                    ps = mmpool.tile([128, P], F32, tag="mm")
                    for kk in range(KT):
                        nc.tensor.matmul(
                            ps[:], lhsT=wkv_sb[:, kk, 128 * m : 128 * (m + 1)],
                            rhs=xk[:, kk, :], start=kk == 0, stop=kk == KT - 1)
                    nc.scalar.activation(kTt[:, m, :], ps[:], AF.Identity,
                                         bias=bkv_sb[:, m : m + 1])
                for i in range(PT):
                    ps = mmpool.tile([128, D], F32, tag="mm")
                    for kk in range(KT):
                        nc.tensor.matmul(
                            ps[:], lhsT=xv[:, kk, 128 * i : 128 * (i + 1)],
                            rhs=wkv_sb[:, kk, :], start=kk == 0, stop=kk == KT - 1)
                    nc.vector.tensor_add(vn[:, i, :], ps[:], bkvb_sb[:])

                eT = apool.tile([128, PT, P], BF16, tag="eT")  # [s, p]
                for i in range(PT):
                    ps = mmpool.tile([128, P], F32, tag="mm")
                    for kk in range(KT):
                        nc.tensor.matmul(
                            ps[:], lhsT=kTt[:, kk, 128 * i : 128 * (i + 1)],
                            rhs=qT[:, kk, :], start=kk == 0, stop=kk == KT - 1)
                    nc.scalar.activation(eT[:, i, :], ps[:], AF.Exp)
                    nc.vector.tensor_mul(
                        eT[:, i, 128 * i : 128 * (i + 1)],
                        eT[:, i, 128 * i : 128 * (i + 1)],
                        mask_sb[:],
                    )

                ps_sum = sumpool.tile([128, PT], F32, tag="sums")
                for j in range(PT):
                    for i in range(PT):
                        nc.tensor.matmul(
                            ps_sum[:, j : j + 1],
                            lhsT=eT[:, i, 128 * j : 128 * (j + 1)],
                            rhs=ones_sb[:], start=i == 0, stop=i == PT - 1)
                rcpT = spool.tile([128, PT], F32, tag="rcpT")
                nc.vector.reciprocal(rcpT[:], ps_sum[:])

                avT = apool.tile([128, KT, P], BF16, tag="avT")  # [dv, p]
                for m in range(KT):
                    ps = mmpool.tile([128, P], F32, tag="mm")
                    for i in range(PT):
                        nc.tensor.matmul(
                            ps[:], lhsT=vn[:, i, 128 * m : 128 * (m + 1)],
                            rhs=eT[:, i, :], start=i == 0, stop=i == PT - 1)
                    nc.scalar.copy(avT[:, m, :], ps[:])

                ot = opool.tile([128, PT, D], F32, tag="ot")
                for j in range(PT):
                    ps = mmpool.tile([128, D], F32, tag="mm")
                    for m in range(KT):
                        nc.tensor.matmul(
                            ps[:], lhsT=avT[:, m, 128 * j : 128 * (j + 1)],
                            rhs=wo_sb[:, m, :], start=m == 0, stop=m == KT - 1)
                    nc.vector.scalar_tensor_tensor(
                        ot[:, j, :], ps[:], rcpT[:, j : j + 1], bob_sb[:],
                        ALU.mult, ALU.add,
                    )
                nc.sync.dma_start(
                    out=out_d.ap()[s].rearrange("(j p) f -> p j f", p=128),
                    in_=ot[:],
                )

    nc.compile()
    return nc


def _bf16(a):
    return np.ascontiguousarray(a).astype(ml_dtypes.bfloat16)


def _norm_inputs(queries, keys, values, Wq, bq, Wkv, bkv, Wo, bo):
    return (
        np.asarray(queries, np.float32).reshape(SLICES, P, D),
        np.asarray(keys, np.float32).reshape(SLICES, P, D),
        np.asarray(values, np.float32).reshape(SLICES, P, D),
        np.asarray(Wq, np.float32), np.asarray(bq, np.float32),
        np.asarray(Wkv, np.float32), np.asarray(bkv, np.float32),
        np.asarray(Wo, np.float32), np.asarray(bo, np.float32),
    )


def prep_in_maps_fast(queries, keys, values, Wq, bq, Wkv, bkv, Wo, bo):
    queries, keys, values, Wq, bq, Wkv, bkv, Wo, bo = _norm_inputs(
        queries, keys, values, Wq, bq, Wkv, bkv, Wo, bo)

    scale = np.float32(1.0 / np.sqrt(D))
    # scores.T = Xk @ M.T @ Xq.T with M[d2,d1] = (scale*Wq).T @ Wkv
    Mh = _bf16((Wq * scale).T @ Wkv)           # [d2, d1]
    N0h = _bf16(Wkv.T @ Wo.T)                  # [d, dout]
    bo2 = Wo @ bkv + bo
    bo2_bc = np.ascontiguousarray(np.broadcast_to(bo2, (128, D))).astype(np.float32)
    mask = _bf16(1.0 - np.eye(128, dtype=np.float32))

    qT = _bf16(queries.transpose(0, 2, 1))     # [slices, D, P]
    kT = _bf16(keys.transpose(0, 2, 1))
    vN = _bf16(values)                         # natural [slices, P, D]

    in_maps = []
    for c in range(N_CORES):
        sl = slice(c * SPC, (c + 1) * SPC)
        in_maps.append({
            "xqT": qT[sl], "xkT": kT[sl], "xvN": vN[sl],
            "Mh": Mh, "N0h": N0h, "bo2_bc": bo2_bc, "mask": mask,
            "ones1": np.ones((128, 1), ml_dtypes.bfloat16),
        })
    return in_maps


def prep_in_maps_general(queries, keys, values, Wq, bq, Wkv, bkv, Wo, bo):
    queries, keys, values, Wq, bq, Wkv, bkv, Wo, bo = _norm_inputs(
        queries, keys, values, Wq, bq, Wkv, bkv, Wo, bo)

    scale = np.float32(1.0 / np.sqrt(D))
    wqT = _bf16((Wq * scale).T)
    wkvT = _bf16(Wkv.T)
    woT = _bf16(Wo.T)
    bq_col = np.ascontiguousarray((bq * scale).reshape(KT, 128).T)
    bkv_col = np.ascontiguousarray(bkv.reshape(KT, 128).T)
    bkv_bc = np.ascontiguousarray(np.broadcast_to(bkv, (128, D))).astype(np.float32)
    bo_bc = np.ascontiguousarray(np.broadcast_to(bo, (128, D))).astype(np.float32)
    mask = _bf16(1.0 - np.eye(128, dtype=np.float32))

    qT = _bf16(queries.transpose(0, 2, 1))
    kT = _bf16(keys.transpose(0, 2, 1))
    vT = _bf16(values.transpose(0, 2, 1))

    in_maps = []
    for c in range(N_CORES):
        sl = slice(c * SPC, (c + 1) * SPC)
        in_maps.append({
            "xqT": qT[sl], "xkT": kT[sl], "xvT": vT[sl],
            "wqT": wqT, "wkvT": wkvT, "woT": woT,
            "bq_col": bq_col, "bkv_col": bkv_col,
            "bkv_bc": bkv_bc, "bo_bc": bo_bc, "mask": mask,
            "ones1": np.ones((128, 1), ml_dtypes.bfloat16),
        })
    return in_maps


_nc_fast = None
_nc_general = None


def kernel(**inputs):
    global _nc_fast, _nc_general
    bq = np.asarray(inputs["bq"], np.float32)
    bkv = np.asarray(inputs["bkv"], np.float32)
    fast = not (np.any(bq) or np.any(bkv))
    if fast:
        if _nc_fast is None:
            _nc_fast = build_program_fast()
        nc, in_maps = _nc_fast, prep_in_maps_fast(**inputs)
    else:
        if _nc_general is None:
            _nc_general = build_program_general()
        nc, in_maps = _nc_general, prep_in_maps_general(**inputs)
    res = run_bass_kernel_spmd(nc, in_maps, core_ids=list(range(N_CORES)))
    out = np.concatenate([res.results[c]["out"] for c in range(N_CORES)], axis=0)
    return out.reshape(B, V, P, D)


# revision 12
# speedup vs baseline: 1.8654x; 1.0092x over previous
"""Trainium2 Bass kernel for per-(b,v)-slice masked attention.

Reference computation (per (b,v) slice, P=S=512, D=512):
    q = X_q @ Wq.T + bq          (softmax scale folded into Wq here)
    k = X_k @ Wkv.T + bkv
    v = X_v @ Wkv.T + bkv
    scores = q @ k.T, diag masked, attn = softmax(scores)
    out = (attn @ v) @ Wo.T + bo

Sharding: 128 (b,v) slices split 16-per-core across 8 cores; weights
replicated. The host pre-transposes q/k activations to d-major layout so
every on-chip matmul contracts over the partition dimension.

Fast path (bq == bkv == 0, which setup_inputs guarantees): fold the
weight products on the host —
    M  = (scale*Wq).T @ Wkv     so  scores.T = Xk @ M.T @ Xq.T
    N0 = Wkv.T @ Wo.T           so  out = (attn @ Xv) @ N0 + (Wo@bkv + bo)
eliminating the k and v projections (4 big matmul groups per slice
instead of 6). Per slice:
    u[d,p]    = (M tiles).T @ XqT
    sT[s,p]   = (XkT tiles).T @ u       (scores transposed)
    eT[s,p]   = exp(sT) * (1 - I)      (diag mask, multiplicative)
    sums[p]   = (eT tiles).T @ ones    (softmax denominator, [128,4] psum)
    axT[d,p]  = (Xv tiles).T @ eT      (attn @ Xv, transposed, unnormalized)
    out[p,o]  = ((axT tiles).T @ N0) * recip(sums)[p] + bo2

General path (nonzero bq/bkv): explicit q/k/v projections as above.

Matmul operands are bf16 (PE streams 1 cycle/row with fast weight load);
all PSUM accumulation and softmax normalization stay fp32.
"""

import numpy as np
import ml_dtypes

import concourse.bacc as bacc
import concourse.mybir as mybir
from concourse.tile import TileContext
from concourse.bass_utils import run_bass_kernel_spmd

B, V, P, D = 4, 32, 512, 512
N_CORES = 8
SLICES = B * V  # 128
SPC = SLICES // N_CORES  # 16 slices per core
KT = D // 128  # 4 contraction tiles
PT = P // 128  # 4 token tiles

BF16 = mybir.dt.bfloat16
F32 = mybir.dt.float32
AF = mybir.ActivationFunctionType
ALU = mybir.AluOpType


def _new_nc():
    return bacc.Bacc("TRN2", target_bir_lowering=False, debug=False,
                     num_devices=N_CORES)


def _load_w(nc, cpool, dram):
    t = cpool.tile([128, KT, D], BF16, tag=dram.name)
    nc.sync.dma_start(out=t[:], in_=dram.ap().rearrange("(kk p) f -> p kk f", p=128))
    return t


def _load_x(nc, xpool, dram, s, tag):
    t = xpool.tile([128, KT, P], BF16, tag=tag)
    nc.sync.dma_start(out=t[:], in_=dram.ap()[s].rearrange("(kk p) f -> p kk f", p=128))
    return t


def build_program_fast():
    """Zero-bias fast path: 4 matmul groups per slice."""
    nc = _new_nc()

    xq_d = nc.dram_tensor("xqT", [SPC, D, P], BF16, kind="ExternalInput")
    xk_d = nc.dram_tensor("xkT", [SPC, D, P], BF16, kind="ExternalInput")
    xv_d = nc.dram_tensor("xvN", [SPC, P, D], BF16, kind="ExternalInput")
    m_d = nc.dram_tensor("Mh", [D, D], BF16, kind="ExternalInput")
    n0_d = nc.dram_tensor("N0h", [D, D], BF16, kind="ExternalInput")
    bo2_d = nc.dram_tensor("bo2_bc", [128, D], F32, kind="ExternalInput")
    mask_d = nc.dram_tensor("mask", [128, 128], BF16, kind="ExternalInput")
    ones_d = nc.dram_tensor("ones1", [128, 1], BF16, kind="ExternalInput")
    out_d = nc.dram_tensor("out", [SPC, P, D], F32, kind="ExternalOutput")

    with TileContext(nc) as tc:
        with (
            tc.tile_pool(name="consts", bufs=1) as cpool,
            tc.tile_pool(name="xin", bufs=2) as xpool,
            tc.tile_pool(name="proj", bufs=2) as ppool,
            tc.tile_pool(name="attn", bufs=2) as apool,
            tc.tile_pool(name="outp", bufs=2) as opool,
            tc.tile_pool(name="small", bufs=2) as spool,
            tc.tile_pool(name="psum", bufs=7, space="PSUM") as mmpool,
            tc.tile_pool(name="psum_sums", bufs=1, space="PSUM") as sumpool,
        ):
            m_sb = _load_w(nc, cpool, m_d)
            n0_sb = _load_w(nc, cpool, n0_d)
            bo2_sb = cpool.tile([128, D], F32, tag="bo2")
            nc.sync.dma_start(out=bo2_sb[:], in_=bo2_d.ap())
            mask_sb = cpool.tile([128, 128], BF16, tag="mask")
            nc.sync.dma_start(out=mask_sb[:], in_=mask_d.ap())
            ones_sb = cpool.tile([128, 1], BF16, tag="ones1")
            nc.sync.dma_start(out=ones_sb[:], in_=ones_d.ap())

            for s in range(SPC):
                xq = _load_x(nc, xpool, xq_d, s, "xq")
                xk = _load_x(nc, xpool, xk_d, s, "xk")
                xv = _load_x(nc, xpool, xv_d, s, "xv")  # natural [s, d] tiles

                # ---- u = M.T-tiles @ XqT ----
                u = ppool.tile([128, KT, P], BF16, tag="u")  # [d1, p]
                for m in range(KT):
                    ps = mmpool.tile([128, P], F32, tag="mm")
                    for kk in range(KT):
                        nc.tensor.matmul(
                            ps[:], lhsT=m_sb[:, kk, 128 * m : 128 * (m + 1)],
                            rhs=xq[:, kk, :], start=kk == 0, stop=kk == KT - 1)
                    nc.scalar.copy(u[:, m, :], ps[:])

                # ---- scoresT + exp + diag mask ----
                eT = apool.tile([128, PT, P], BF16, tag="eT")  # [s, p]
                for i in range(PT):
                    ps = mmpool.tile([128, P], F32, tag="mm")
                    for kk in range(KT):
                        nc.tensor.matmul(
                            ps[:], lhsT=xk[:, kk, 128 * i : 128 * (i + 1)],
                            rhs=u[:, kk, :], start=kk == 0, stop=kk == KT - 1)
                    nc.scalar.activation(eT[:, i, :], ps[:], AF.Exp)
                    nc.vector.tensor_mul(
                        eT[:, i, 128 * i : 128 * (i + 1)],
                        eT[:, i, 128 * i : 128 * (i + 1)],
                        mask_sb[:],
                    )

                # ---- softmax denominators ----
                ps_sum = sumpool.tile([128, PT], F32, tag="sums")
                for j in range(PT):
                    for i in range(PT):
                        nc.tensor.matmul(
                            ps_sum[:, j : j + 1],
                            lhsT=eT[:, i, 128 * j : 128 * (j + 1)],
                            rhs=ones_sb[:], start=i == 0, stop=i == PT - 1)
                rcpT = spool.tile([128, PT], F32, tag="rcpT")
                nc.vector.reciprocal(rcpT[:], ps_sum[:])

                # ---- axT = attn @ Xv, transposed (unnormalized) ----
                axT = apool.tile([128, KT, P], BF16, tag="axT")  # [d, p]
                for m in range(KT):
                    ps = mmpool.tile([128, P], F32, tag="mm")
                    for i in range(PT):
                        nc.tensor.matmul(
                            ps[:], lhsT=xv[:, i, 128 * m : 128 * (m + 1)],
                            rhs=eT[:, i, :], start=i == 0, stop=i == PT - 1)
                    nc.scalar.copy(axT[:, m, :], ps[:])

                # ---- final projection + normalize + bias ----
                ot = opool.tile([128, PT, D], F32, tag="ot")
                for j in range(PT):
                    ps = mmpool.tile([128, D], F32, tag="mm")
                    for m in range(KT):
                        nc.tensor.matmul(
                            ps[:], lhsT=axT[:, m, 128 * j : 128 * (j + 1)],
                            rhs=n0_sb[:, m, :], start=m == 0, stop=m == KT - 1)
                    nc.vector.scalar_tensor_tensor(
                        ot[:, j, :], ps[:], rcpT[:, j : j + 1], bo2_sb[:],
                        ALU.mult, ALU.add,
                    )
                    # per-j store so the final DMA overlaps the epilogue
                    nc.sync.dma_start(
                        out=out_d.ap()[s, 128 * j : 128 * (j + 1), :],
                        in_=ot[:, j, :],
                    )

    nc.compile()
    return nc


def build_program_general():
    """General path with explicit q/k/v projections (nonzero biases)."""
    nc = _new_nc()

    xq_d = nc.dram_tensor("xqT", [SPC, D, P], BF16, kind="ExternalInput")
    xk_d = nc.dram_tensor("xkT", [SPC, D, P], BF16, kind="ExternalInput")
    xv_d = nc.dram_tensor("xvT", [SPC, D, P], BF16, kind="ExternalInput")
    wq_d = nc.dram_tensor("wqT", [D, D], BF16, kind="ExternalInput")
    wkv_d = nc.dram_tensor("wkvT", [D, D], BF16, kind="ExternalInput")
    wo_d = nc.dram_tensor("woT", [D, D], BF16, kind="ExternalInput")
    bq_d = nc.dram_tensor("bq_col", [128, KT], F32, kind="ExternalInput")
    bkv_d = nc.dram_tensor("bkv_col", [128, KT], F32, kind="ExternalInput")
    bkvb_d = nc.dram_tensor("bkv_bc", [128, D], F32, kind="ExternalInput")
    bob_d = nc.dram_tensor("bo_bc", [128, D], F32, kind="ExternalInput")
    mask_d = nc.dram_tensor("mask", [128, 128], BF16, kind="ExternalInput")
    ones_d = nc.dram_tensor("ones1", [128, 1], BF16, kind="ExternalInput")
    out_d = nc.dram_tensor("out", [SPC, P, D], F32, kind="ExternalOutput")

    with TileContext(nc) as tc:
        with (
            tc.tile_pool(name="consts", bufs=1) as cpool,
            tc.tile_pool(name="xin", bufs=2) as xpool,
            tc.tile_pool(name="proj", bufs=2) as ppool,
            tc.tile_pool(name="attn", bufs=2) as apool,
            tc.tile_pool(name="outp", bufs=2) as opool,
            tc.tile_pool(name="small", bufs=2) as spool,
            tc.tile_pool(name="psum", bufs=6, space="PSUM") as mmpool,
            tc.tile_pool(name="psum_sums", bufs=2, space="PSUM") as sumpool,
        ):
            wq_sb = _load_w(nc, cpool, wq_d)
            wkv_sb = _load_w(nc, cpool, wkv_d)
            wo_sb = _load_w(nc, cpool, wo_d)
            bq_sb = cpool.tile([128, KT], F32, tag="bq")
            nc.sync.dma_start(out=bq_sb[:], in_=bq_d.ap())
            bkv_sb = cpool.tile([128, KT], F32, tag="bkv")
            nc.sync.dma_start(out=bkv_sb[:], in_=bkv_d.ap())
            bkvb_sb = cpool.tile([128, D], F32, tag="bkvb")
            nc.sync.dma_start(out=bkvb_sb[:], in_=bkvb_d.ap())
            bob_sb = cpool.tile([128, D], F32, tag="bob")
            nc.sync.dma_start(out=bob_sb[:], in_=bob_d.ap())
            mask_sb = cpool.tile([128, 128], BF16, tag="mask")
            nc.sync.dma_start(out=mask_sb[:], in_=mask_d.ap())
            ones_sb = cpool.tile([128, 1], BF16, tag="ones1")
            nc.sync.dma_start(out=ones_sb[:], in_=ones_d.ap())

            for s in range(SPC):
                xq = _load_x(nc, xpool, xq_d, s, "xq")
                xk = _load_x(nc, xpool, xk_d, s, "xk")
                xv = _load_x(nc, xpool, xv_d, s, "xv")

                qT = ppool.tile([128, KT, P], BF16, tag="qT")  # [dout, p]
                kTt = ppool.tile([128, KT, P], BF16, tag="kT")  # [dout, s]
                vn = ppool.tile([128, PT, D], BF16, tag="vn")  # [s, dout]
                for m in range(KT):
                    ps = mmpool.tile([128, P], F32, tag="mm")
                    for kk in range(KT):
                        nc.tensor.matmul(
                            ps[:], lhsT=wq_sb[:, kk, 128 * m : 128 * (m + 1)],
                            rhs=xq[:, kk, :], start=kk == 0, stop=kk == KT - 1)
                    nc.scalar.activation(qT[:, m, :], ps[:], AF.Identity,
                                         bias=bq_sb[:, m : m + 1])
                for m in range(KT):
                    ps = mmpool.tile([128, P], F32, tag="mm")
                    for kk in range(KT):
                        nc.tensor.matmul(
                            ps[:], lhsT=wkv_sb[:, kk, 128 * m : 128 * (m + 1)],
                            rhs=xk[:, kk, :], start=kk == 0, stop=kk == KT - 1)
                    nc.scalar.activation(kTt[:, m, :], ps[:], AF.Identity,
                                         bias=bkv_sb[:, m : m + 1])
                for i in range(PT):
                    ps = mmpool.tile([128, D], F32, tag="mm")
                    for kk in range(KT):
                        nc.tensor.matmul(
                            ps[:], lhsT=xv[:, kk, 128 * i : 128 * (i + 1)],
                            rhs=wkv_sb[:, kk, :], start=kk == 0, stop=kk == KT - 1)
                    nc.vector.tensor_add(vn[:, i, :], ps[:], bkvb_sb[:])

                eT = apool.tile([128, PT, P], BF16, tag="eT")  # [s, p]
                for i in range(PT):
                    ps = mmpool.tile([128, P], F32, tag="mm")
                    for kk in range(KT):
                        nc.tensor.matmul(
                            ps[:], lhsT=kTt[:, kk, 128 * i : 128 * (i + 1)],
                            rhs=qT[:, kk, :], start=kk == 0, stop=kk == KT - 1)
                    nc.scalar.activation(eT[:, i, :], ps[:], AF.Exp)
                    nc.vector.tensor_mul(
                        eT[:, i, 128 * i : 128 * (i + 1)],
                        eT[:, i, 128 * i : 128 * (i + 1)],
                        mask_sb[:],
                    )

                ps_sum = sumpool.tile([128, PT], F32, tag="sums")
                for j in range(PT):
                    for i in range(PT):
                        nc.tensor.matmul(
                            ps_sum[:, j : j + 1],
                            lhsT=eT[:, i, 128 * j : 128 * (j + 1)],
                            rhs=ones_sb[:], start=i == 0, stop=i == PT - 1)
                rcpT = spool.tile([128, PT], F32, tag="rcpT")
                nc.vector.reciprocal(rcpT[:], ps_sum[:])

                avT = apool.tile([128, KT, P], BF16, tag="avT")  # [dv, p]
                for m in range(KT):
                    ps = mmpool.tile([128, P], F32, tag="mm")
                    for i in range(PT):
                        nc.tensor.matmul(
                            ps[:], lhsT=vn[:, i, 128 * m : 128 * (m + 1)],
                            rhs=eT[:, i, :], start=i == 0, stop=i == PT - 1)
                    nc.scalar.copy(avT[:, m, :], ps[:])

                ot = opool.tile([128, PT, D], F32, tag="ot")
                for j in range(PT):
                    ps = mmpool.tile([128, D], F32, tag="mm")
                    for m in range(KT):
                        nc.tensor.matmul(
                            ps[:], lhsT=avT[:, m, 128 * j : 128 * (j + 1)],
                            rhs=wo_sb[:, m, :], start=m == 0, stop=m == KT - 1)
                    nc.vector.scalar_tensor_tensor(
                        ot[:, j, :], ps[:], rcpT[:, j : j + 1], bob_sb[:],
                        ALU.mult, ALU.add,
                    )
                nc.sync.dma_start(
                    out=out_d.ap()[s].rearrange("(j p) f -> p j f", p=128),
                    in_=ot[:],
                )

    nc.compile()
    return nc


def _bf16(a):
    return np.ascontiguousarray(a).astype(ml_dtypes.bfloat16)


def _norm_inputs(queries, keys, values, Wq, bq, Wkv, bkv, Wo, bo):
    return (
        np.asarray(queries, np.float32).reshape(SLICES, P, D),
        np.asarray(keys, np.float32).reshape(SLICES, P, D),
        np.asarray(values, np.float32).reshape(SLICES, P, D),
        np.asarray(Wq, np.float32), np.asarray(bq, np.float32),
        np.asarray(Wkv, np.float32), np.asarray(bkv, np.float32),
        np.asarray(Wo, np.float32), np.asarray(bo, np.float32),
    )


def prep_in_maps_fast(queries, keys, values, Wq, bq, Wkv, bkv, Wo, bo):
    queries, keys, values, Wq, bq, Wkv, bkv, Wo, bo = _norm_inputs(
        queries, keys, values, Wq, bq, Wkv, bkv, Wo, bo)

    scale = np.float32(1.0 / np.sqrt(D))
    # scores.T = Xk @ M.T @ Xq.T with M[d2,d1] = (scale*Wq).T @ Wkv
    Mh = _bf16((Wq * scale).T @ Wkv)           # [d2, d1]
    N0h = _bf16(Wkv.T @ Wo.T)                  # [d, dout]
    bo2 = Wo @ bkv + bo
    bo2_bc = np.ascontiguousarray(np.broadcast_to(bo2, (128, D))).astype(np.float32)
    mask = _bf16(1.0 - np.eye(128, dtype=np.float32))

    qT = _bf16(queries.transpose(0, 2, 1))     # [slices, D, P]
    kT = _bf16(keys.transpose(0, 2, 1))
    vN = _bf16(values)                         # natural [slices, P, D]

    in_maps = []
    for c in range(N_CORES):
        sl = slice(c * SPC, (c + 1) * SPC)
        in_maps.append({
            "xqT": qT[sl], "xkT": kT[sl], "xvN": vN[sl],
            "Mh": Mh, "N0h": N0h, "bo2_bc": bo2_bc, "mask": mask,
            "ones1": np.ones((128, 1), ml_dtypes.bfloat16),
        })
    return in_maps


def prep_in_maps_general(queries, keys, values, Wq, bq, Wkv, bkv, Wo, bo):
    queries, keys, values, Wq, bq, Wkv, bkv, Wo, bo = _norm_inputs(
        queries, keys, values, Wq, bq, Wkv, bkv, Wo, bo)

    scale = np.float32(1.0 / np.sqrt(D))
    wqT = _bf16((Wq * scale).T)
    wkvT = _bf16(Wkv.T)
    woT = _bf16(Wo.T)
    bq_col = np.ascontiguousarray((bq * scale).reshape(KT, 128).T)
    bkv_col = np.ascontiguousarray(bkv.reshape(KT, 128).T)
    bkv_bc = np.ascontiguousarray(np.broadcast_to(bkv, (128, D))).astype(np.float32)
    bo_bc = np.ascontiguousarray(np.broadcast_to(bo, (128, D))).astype(np.float32)
    mask = _bf16(1.0 - np.eye(128, dtype=np.float32))

    qT = _bf16(queries.transpose(0, 2, 1))
    kT = _bf16(keys.transpose(0, 2, 1))
    vT = _bf16(values.transpose(0, 2, 1))

    in_maps = []
    for c in range(N_CORES):
        sl = slice(c * SPC, (c + 1) * SPC)
        in_maps.append({
            "xqT": qT[sl], "xkT": kT[sl], "xvT": vT[sl],
            "wqT": wqT, "wkvT": wkvT, "woT": woT,
            "bq_col": bq_col, "bkv_col": bkv_col,
            "bkv_bc": bkv_bc, "bo_bc": bo_bc, "mask": mask,
            "ones1": np.ones((128, 1), ml_dtypes.bfloat16),
        })
    return in_maps


_nc_fast = None
_nc_general = None


def kernel(**inputs):
    global _nc_fast, _nc_general
    bq = np.asarray(inputs["bq"], np.float32)
    bkv = np.asarray(inputs["bkv"], np.float32)
    fast = not (np.any(bq) or np.any(bkv))
    if fast:
        if _nc_fast is None:
            _nc_fast = build_program_fast()
        nc, in_maps = _nc_fast, prep_in_maps_fast(**inputs)
    else:
        if _nc_general is None:
            _nc_general = build_program_general()
        nc, in_maps = _nc_general, prep_in_maps_general(**inputs)
    res = run_bass_kernel_spmd(nc, in_maps, core_ids=list(range(N_CORES)))
    out = np.concatenate([res.results[c]["out"] for c in range(N_CORES)], axis=0)
    return out.reshape(B, V, P, D)


# revision 13
# speedup vs baseline: 1.8790x; 1.0073x over previous
"""Trainium2 Bass kernel for per-(b,v)-slice masked attention.

Reference computation (per (b,v) slice, P=S=512, D=512):
    q = X_q @ Wq.T + bq          (softmax scale folded into Wq here)
    k = X_k @ Wkv.T + bkv
    v = X_v @ Wkv.T + bkv
    scores = q @ k.T, diag masked, attn = softmax(scores)
    out = (attn @ v) @ Wo.T + bo

Sharding: 128 (b,v) slices split 16-per-core across 8 cores; weights
replicated. The host pre-transposes q/k activations to d-major layout so
every on-chip matmul contracts over the partition dimension.

Fast path (bq == bkv == 0, which setup_inputs guarantees): fold the
weight products on the host —
    M  = (scale*Wq).T @ Wkv     so  scores.T = Xk @ M.T @ Xq.T
    N0 = Wkv.T @ Wo.T           so  out = (attn @ Xv) @ N0 + (Wo@bkv + bo)
eliminating the k and v projections (4 big matmul groups per slice
instead of 6). Per slice:
    u[d,p]    = (M tiles).T @ XqT
    sT[s,p]   = (XkT tiles).T @ u       (scores transposed)
    eT[s,p]   = exp(sT) * (1 - I)      (diag mask, multiplicative)
    sums[p]   = (eT tiles).T @ ones    (softmax denominator, [128,4] psum)
    axT[d,p]  = (Xv tiles).T @ eT      (attn @ Xv, transposed, unnormalized)
    out[p,o]  = ((axT tiles).T @ N0) * recip(sums)[p] + bo2

General path (nonzero bq/bkv): explicit q/k/v projections as above.

Matmul operands are bf16 (PE streams 1 cycle/row with fast weight load);
all PSUM accumulation and softmax normalization stay fp32.
"""

import numpy as np
import ml_dtypes

import concourse.bacc as bacc
import concourse.mybir as mybir
from concourse.tile import TileContext
from concourse.bass_utils import run_bass_kernel_spmd

B, V, P, D = 4, 32, 512, 512
N_CORES = 8
SLICES = B * V  # 128
SPC = SLICES // N_CORES  # 16 slices per core
KT = D // 128  # 4 contraction tiles
PT = P // 128  # 4 token tiles

BF16 = mybir.dt.bfloat16
F32 = mybir.dt.float32
AF = mybir.ActivationFunctionType
ALU = mybir.AluOpType


def _new_nc():
    return bacc.Bacc("TRN2", target_bir_lowering=False, debug=False,
                     num_devices=N_CORES)


def _load_w(nc, cpool, dram):
    t = cpool.tile([128, KT, D], BF16, tag=dram.name)
    nc.sync.dma_start(out=t[:], in_=dram.ap().rearrange("(kk p) f -> p kk f", p=128))
    return t


def _load_x(nc, xpool, dram, s, tag):
    t = xpool.tile([128, KT, P], BF16, tag=tag)
    nc.sync.dma_start(out=t[:], in_=dram.ap()[s].rearrange("(kk p) f -> p kk f", p=128))
    return t


def build_program_fast():
    """Zero-bias fast path: 4 matmul groups per slice."""
    nc = _new_nc()

    xq_d = nc.dram_tensor("xqT", [SPC, D, P], BF16, kind="ExternalInput")
    xk_d = nc.dram_tensor("xkT", [SPC, D, P], BF16, kind="ExternalInput")
    xv_d = nc.dram_tensor("xvN", [SPC, P, D], BF16, kind="ExternalInput")
    m_d = nc.dram_tensor("Mh", [D, D], BF16, kind="ExternalInput")
    n0_d = nc.dram_tensor("N0h", [D, D], BF16, kind="ExternalInput")
    bo2_d = nc.dram_tensor("bo2_bc", [128, D], F32, kind="ExternalInput")
    mask_d = nc.dram_tensor("mask", [128, 128], BF16, kind="ExternalInput")
    ones_d = nc.dram_tensor("ones1", [128, 1], BF16, kind="ExternalInput")
    out_d = nc.dram_tensor("out", [SPC, P, D], F32, kind="ExternalOutput")

    with TileContext(nc) as tc:
        with (
            tc.tile_pool(name="consts", bufs=1) as cpool,
            tc.tile_pool(name="xin", bufs=2) as xpool,
            tc.tile_pool(name="proj", bufs=2) as ppool,
            tc.tile_pool(name="attn", bufs=2) as apool,
            tc.tile_pool(name="outp", bufs=2) as opool,
            tc.tile_pool(name="small", bufs=2) as spool,
            tc.tile_pool(name="psum", bufs=7, space="PSUM") as mmpool,
            tc.tile_pool(name="psum_sums", bufs=1, space="PSUM") as sumpool,
        ):
            # Mh gates the first matmul — load it (and the small consts)
            # first; N0h/bo2 are consumed late, so they load after slice 0's
            # activations to keep DMA bandwidth on the critical startup path.
            m_sb = _load_w(nc, cpool, m_d)
            mask_sb = cpool.tile([128, 128], BF16, tag="mask")
            nc.sync.dma_start(out=mask_sb[:], in_=mask_d.ap())
            ones_sb = cpool.tile([128, 1], BF16, tag="ones1")
            nc.sync.dma_start(out=ones_sb[:], in_=ones_d.ap())

            for s in range(SPC):
                xq = _load_x(nc, xpool, xq_d, s, "xq")
                xk = _load_x(nc, xpool, xk_d, s, "xk")
                xv = _load_x(nc, xpool, xv_d, s, "xv")  # natural [s, d] tiles
                if s == 0:
                    n0_sb = _load_w(nc, cpool, n0_d)
                    bo2_sb = cpool.tile([128, D], F32, tag="bo2")
                    nc.sync.dma_start(out=bo2_sb[:], in_=bo2_d.ap())

                # ---- u = M.T-tiles @ XqT ----
                u = ppool.tile([128, KT, P], BF16, tag="u")  # [d1, p]
                for m in range(KT):
                    ps = mmpool.tile([128, P], F32, tag="mm")
                    for kk in range(KT):
                        nc.tensor.matmul(
                            ps[:], lhsT=m_sb[:, kk, 128 * m : 128 * (m + 1)],
                            rhs=xq[:, kk, :], start=kk == 0, stop=kk == KT - 1)
                    nc.scalar.copy(u[:, m, :], ps[:])

                # ---- scoresT + exp + diag mask ----
                eT = apool.tile([128, PT, P], BF16, tag="eT")  # [s, p]
                for i in range(PT):
                    ps = mmpool.tile([128, P], F32, tag="mm")
                    for kk in range(KT):
                        nc.tensor.matmul(
                            ps[:], lhsT=xk[:, kk, 128 * i : 128 * (i + 1)],
                            rhs=u[:, kk, :], start=kk == 0, stop=kk == KT - 1)
                    nc.scalar.activation(eT[:, i, :], ps[:], AF.Exp)
                    nc.vector.tensor_mul(
                        eT[:, i, 128 * i : 128 * (i + 1)],
                        eT[:, i, 128 * i : 128 * (i + 1)],
                        mask_sb[:],
                    )

                # ---- softmax denominators ----
                ps_sum = sumpool.tile([128, PT], F32, tag="sums")
                for j in range(PT):
                    for i in range(PT):
                        nc.tensor.matmul(
                            ps_sum[:, j : j + 1],
                            lhsT=eT[:, i, 128 * j : 128 * (j + 1)],
                            rhs=ones_sb[:], start=i == 0, stop=i == PT - 1)
                rcpT = spool.tile([128, PT], F32, tag="rcpT")
                nc.vector.reciprocal(rcpT[:], ps_sum[:])

                # ---- axT = attn @ Xv, transposed (unnormalized) ----
                axT = apool.tile([128, KT, P], BF16, tag="axT")  # [d, p]
                for m in range(KT):
                    ps = mmpool.tile([128, P], F32, tag="mm")
                    for i in range(PT):
                        nc.tensor.matmul(
                            ps[:], lhsT=xv[:, i, 128 * m : 128 * (m + 1)],
                            rhs=eT[:, i, :], start=i == 0, stop=i == PT - 1)
                    nc.scalar.copy(axT[:, m, :], ps[:])

                # ---- final projection + normalize + bias ----
                ot = opool.tile([128, PT, D], F32, tag="ot")
                for j in range(PT):
                    ps = mmpool.tile([128, D], F32, tag="mm")
                    for m in range(KT):
                        nc.tensor.matmul(
                            ps[:], lhsT=axT[:, m, 128 * j : 128 * (j + 1)],
                            rhs=n0_sb[:, m, :], start=m == 0, stop=m == KT - 1)
                    nc.vector.scalar_tensor_tensor(
                        ot[:, j, :], ps[:], rcpT[:, j : j + 1], bo2_sb[:],
                        ALU.mult, ALU.add,
                    )
                    # per-j store so the final DMA overlaps the epilogue
                    nc.sync.dma_start(
                        out=out_d.ap()[s, 128 * j : 128 * (j + 1), :],
                        in_=ot[:, j, :],
                    )

    nc.compile()
    return nc


def build_program_general():
    """General path with explicit q/k/v projections (nonzero biases)."""
    nc = _new_nc()

    xq_d = nc.dram_tensor("xqT", [SPC, D, P], BF16, kind="ExternalInput")
    xk_d = nc.dram_tensor("xkT", [SPC, D, P], BF16, kind="ExternalInput")
    xv_d = nc.dram_tensor("xvT", [SPC, D, P], BF16, kind="ExternalInput")
    wq_d = nc.dram_tensor("wqT", [D, D], BF16, kind="ExternalInput")
    wkv_d = nc.dram_tensor("wkvT", [D, D], BF16, kind="ExternalInput")
    wo_d = nc.dram_tensor("woT", [D, D], BF16, kind="ExternalInput")
    bq_d = nc.dram_tensor("bq_col", [128, KT], F32, kind="ExternalInput")
    bkv_d = nc.dram_tensor("bkv_col", [128, KT], F32, kind="ExternalInput")
    bkvb_d = nc.dram_tensor("bkv_bc", [128, D], F32, kind="ExternalInput")
    bob_d = nc.dram_tensor("bo_bc", [128, D], F32, kind="ExternalInput")
    mask_d = nc.dram_tensor("mask", [128, 128], BF16, kind="ExternalInput")
    ones_d = nc.dram_tensor("ones1", [128, 1], BF16, kind="ExternalInput")
    out_d = nc.dram_tensor("out", [SPC, P, D], F32, kind="ExternalOutput")

    with TileContext(nc) as tc:
        with (
            tc.tile_pool(name="consts", bufs=1) as cpool,
            tc.tile_pool(name="xin", bufs=2) as xpool,
            tc.tile_pool(name="proj", bufs=2) as ppool,
            tc.tile_pool(name="attn", bufs=2) as apool,
            tc.tile_pool(name="outp", bufs=2) as opool,
            tc.tile_pool(name="small", bufs=2) as spool,
            tc.tile_pool(name="psum", bufs=6, space="PSUM") as mmpool,
            tc.tile_pool(name="psum_sums", bufs=2, space="PSUM") as sumpool,
        ):
            wq_sb = _load_w(nc, cpool, wq_d)
            wkv_sb = _load_w(nc, cpool, wkv_d)
            wo_sb = _load_w(nc, cpool, wo_d)
            bq_sb = cpool.tile([128, KT], F32, tag="bq")
            nc.sync.dma_start(out=bq_sb[:], in_=bq_d.ap())
            bkv_sb = cpool.tile([128, KT], F32, tag="bkv")
            nc.sync.dma_start(out=bkv_sb[:], in_=bkv_d.ap())
            bkvb_sb = cpool.tile([128, D], F32, tag="bkvb")
            nc.sync.dma_start(out=bkvb_sb[:], in_=bkvb_d.ap())
            bob_sb = cpool.tile([128, D], F32, tag="bob")
            nc.sync.dma_start(out=bob_sb[:], in_=bob_d.ap())
            mask_sb = cpool.tile([128, 128], BF16, tag="mask")
            nc.sync.dma_start(out=mask_sb[:], in_=mask_d.ap())
            ones_sb = cpool.tile([128, 1], BF16, tag="ones1")
            nc.sync.dma_start(out=ones_sb[:], in_=ones_d.ap())

            for s in range(SPC):
                xq = _load_x(nc, xpool, xq_d, s, "xq")
                xk = _load_x(nc, xpool, xk_d, s, "xk")
                xv = _load_x(nc, xpool, xv_d, s, "xv")

                qT = ppool.tile([128, KT, P], BF16, tag="qT")  # [dout, p]
                kTt = ppool.tile([128, KT, P], BF16, tag="kT")  # [dout, s]
                vn = ppool.tile([128, PT, D], BF16, tag="vn")  # [s, dout]
                for m in range(KT):
                    ps = mmpool.tile([128, P], F32, tag="mm")
                    for kk in range(KT):
                        nc.tensor.matmul(
                            ps[:], lhsT=wq_sb[:, kk, 128 * m : 128 * (m + 1)],
                            rhs=xq[:, kk, :], start=kk == 0, stop=kk == KT - 1)
                    nc.scalar.activation(qT[:, m, :], ps[:], AF.Identity,
                                         bias=bq_sb[:, m : m + 1])
                for m in range(KT):
                    ps = mmpool.tile([128, P], F32, tag="mm")
                    for kk in range(KT):
                        nc.tensor.matmul(
                            ps[:], lhsT=wkv_sb[:, kk, 128 * m : 128 * (m + 1)],
                            rhs=xk[:, kk, :], start=kk == 0, stop=kk == KT - 1)
                    nc.scalar.activation(kTt[:, m, :], ps[:], AF.Identity,
                                         bias=bkv_sb[:, m : m + 1])
                for i in range(PT):
                    ps = mmpool.tile([128, D], F32, tag="mm")
                    for kk in range(KT):
                        nc.tensor.matmul(
                            ps[:], lhsT=xv[:, kk, 128 * i : 128 * (i + 1)],
                            rhs=wkv_sb[:, kk, :], start=kk == 0, stop=kk == KT - 1)
                    nc.vector.tensor_add(vn[:, i, :], ps[:], bkvb_sb[:])

                eT = apool.tile([128, PT, P], BF16, tag="eT")  # [s, p]
                for i in range(PT):
                    ps = mmpool.tile([128, P], F32, tag="mm")
                    for kk in range(KT):
                        nc.tensor.matmul(
                            ps[:], lhsT=kTt[:, kk, 128 * i : 128 * (i + 1)],
                            rhs=qT[:, kk, :], start=kk == 0, stop=kk == KT - 1)
                    nc.scalar.activation(eT[:, i, :], ps[:], AF.Exp)
                    nc.vector.tensor_mul(
                        eT[:, i, 128 * i : 128 * (i + 1)],
                        eT[:, i, 128 * i : 128 * (i + 1)],
                        mask_sb[:],
                    )

                ps_sum = sumpool.tile([128, PT], F32, tag="sums")
                for j in range(PT):
                    for i in range(PT):
                        nc.tensor.matmul(
                            ps_sum[:, j : j + 1],
                            lhsT=eT[:, i, 128 * j : 128 * (j + 1)],
                            rhs=ones_sb[:], start=i == 0, stop=i == PT - 1)
                rcpT = spool.tile([128, PT], F32, tag="rcpT")
                nc.vector.reciprocal(rcpT[:], ps_sum[:])

                avT = apool.tile([128, KT, P], BF16, tag="avT")  # [dv, p]
                for m in range(KT):
                    ps = mmpool.tile([128, P], F32, tag="mm")
                    for i in range(PT):
                        nc.tensor.matmul(
                            ps[:], lhsT=vn[:, i, 128 * m : 128 * (m + 1)],
                            rhs=eT[:, i, :], start=i == 0, stop=i == PT - 1)
                    nc.scalar.copy(avT[:, m, :], ps[:])

                ot = opool.tile([128, PT, D], F32, tag="ot")
                for j in range(PT):
                    ps = mmpool.tile([128, D], F32, tag="mm")
                    for m in range(KT):
                        nc.tensor.matmul(
                            ps[:], lhsT=avT[:, m, 128 * j : 128 * (j + 1)],
                            rhs=wo_sb[:, m, :], start=m == 0, stop=m == KT - 1)
                    nc.vector.scalar_tensor_tensor(
                        ot[:, j, :], ps[:], rcpT[:, j : j + 1], bob_sb[:],
                        ALU.mult, ALU.add,
                    )
                nc.sync.dma_start(
                    out=out_d.ap()[s].rearrange("(j p) f -> p j f", p=128),
                    in_=ot[:],
                )

    nc.compile()
    return nc


def _bf16(a):
    return np.ascontiguousarray(a).astype(ml_dtypes.bfloat16)


def _norm_inputs(queries, keys, values, Wq, bq, Wkv, bkv, Wo, bo):
    return (
        np.asarray(queries, np.float32).reshape(SLICES, P, D),
        np.asarray(keys, np.float32).reshape(SLICES, P, D),
        np.asarray(values, np.float32).reshape(SLICES, P, D),
        np.asarray(Wq, np.float32), np.asarray(bq, np.float32),
        np.asarray(Wkv, np.float32), np.asarray(bkv, np.float32),
        np.asarray(Wo, np.float32), np.asarray(bo, np.float32),
    )


def prep_in_maps_fast(queries, keys, values, Wq, bq, Wkv, bkv, Wo, bo):
    queries, keys, values, Wq, bq, Wkv, bkv, Wo, bo = _norm_inputs(
        queries, keys, values, Wq, bq, Wkv, bkv, Wo, bo)

    scale = np.float32(1.0 / np.sqrt(D))
    # scores.T = Xk @ M.T @ Xq.T with M[d2,d1] = (scale*Wq).T @ Wkv
    Mh = _bf16((Wq * scale).T @ Wkv)           # [d2, d1]
    N0h = _bf16(Wkv.T @ Wo.T)                  # [d, dout]
    bo2 = Wo @ bkv + bo
    bo2_bc = np.ascontiguousarray(np.broadcast_to(bo2, (128, D))).astype(np.float32)
    mask = _bf16(1.0 - np.eye(128, dtype=np.float32))

    qT = _bf16(queries.transpose(0, 2, 1))     # [slices, D, P]
    kT = _bf16(keys.transpose(0, 2, 1))
    vN = _bf16(values)                         # natural [slices, P, D]

    in_maps = []
    for c in range(N_CORES):
        sl = slice(c * SPC, (c + 1) * SPC)
        in_maps.append({
            "xqT": qT[sl], "xkT": kT[sl], "xvN": vN[sl],
            "Mh": Mh, "N0h": N0h, "bo2_bc": bo2_bc, "mask": mask,
            "ones1": np.ones((128, 1), ml_dtypes.bfloat16),
        })
    return in_maps


def prep_in_maps_general(queries, keys, values, Wq, bq, Wkv, bkv, Wo, bo):
    queries, keys, values, Wq, bq, Wkv, bkv, Wo, bo = _norm_inputs(
        queries, keys, values, Wq, bq, Wkv, bkv, Wo, bo)

    scale = np.float32(1.0 / np.sqrt(D))
    wqT = _bf16((Wq * scale).T)
    wkvT = _bf16(Wkv.T)
    woT = _bf16(Wo.T)
    bq_col = np.ascontiguousarray((bq * scale).reshape(KT, 128).T)
    bkv_col = np.ascontiguousarray(bkv.reshape(KT, 128).T)
    bkv_bc = np.ascontiguousarray(np.broadcast_to(bkv, (128, D))).astype(np.float32)
    bo_bc = np.ascontiguousarray(np.broadcast_to(bo, (128, D))).astype(np.float32)
    mask = _bf16(1.0 - np.eye(128, dtype=np.float32))

    qT = _bf16(queries.transpose(0, 2, 1))
    kT = _bf16(keys.transpose(0, 2, 1))
    vT = _bf16(values.transpose(0, 2, 1))

    in_maps = []
    for c in range(N_CORES):
        sl = slice(c * SPC, (c + 1) * SPC)
        in_maps.append({
            "xqT": qT[sl], "xkT": kT[sl], "xvT": vT[sl],
            "wqT": wqT, "wkvT": wkvT, "woT": woT,
            "bq_col": bq_col, "bkv_col": bkv_col,
            "bkv_bc": bkv_bc, "bo_bc": bo_bc, "mask": mask,
            "ones1": np.ones((128, 1), ml_dtypes.bfloat16),
        })
    return in_maps


_nc_fast = None
_nc_general = None


def kernel(**inputs):
    global _nc_fast, _nc_general
    bq = np.asarray(inputs["bq"], np.float32)
    bkv = np.asarray(inputs["bkv"], np.float32)
    fast = not (np.any(bq) or np.any(bkv))
    if fast:
        if _nc_fast is None:
            _nc_fast = build_program_fast()
        nc, in_maps = _nc_fast, prep_in_maps_fast(**inputs)
    else:
        if _nc_general is None:
            _nc_general = build_program_general()
        nc, in_maps = _nc_general, prep_in_maps_general(**inputs)
    res = run_bass_kernel_spmd(nc, in_maps, core_ids=list(range(N_CORES)))
    out = np.concatenate([res.results[c]["out"] for c in range(N_CORES)], axis=0)
    return out.reshape(B, V, P, D)
